# revision 47
# baseline (speedup 1.0000x reference)
"""Trainium2 Bass kernel v3 for nn_CategoricalDecoder (topk_masking).

Phase A (bin-sharded, single-term f32r): tail-feature logits for the local
1024-bin shard, scores packed as int32 (score<<13 | global bin id), local
top-8 per batch row via max8. AllToAll flips to batch sharding (8KB).
Phase B: merge 64 candidates/row -> top-16 packed (ids come free via
bitwise AND), dma_gather of winner z rows from DRAM, single-term f32r
recompute of num/den (den exact via host-folded oh@W2 matmuls), logsumexp.
"""

import numpy as np
from contextlib import ExitStack

import bass_rust as _br
import concourse.bass as bass
import concourse.bacc as bacc
import concourse.tile as tile
from concourse import mybir
from concourse.bass_utils import run_bass_kernel_spmd
from concourse.hw_specs import get_activation_tables

F32 = mybir.dt.float32
F32R = mybir.dt.float32r
I32 = mybir.dt.int32
I16 = mybir.dt.int16
AF = mybir.ActivationFunctionType
ALU = mybir.AluOpType
AX = mybir.AxisListType

B, N, Lz, H, D, C = 256, 8192, 64, 256, 32, 16
DC = D * C
P = 8
NL = N // P
BL = B // P
K = 16
NTAIL = 64  # tail-feature logit rows (4 features x 16 classes)

# pk64 column offsets ([64, C64])
O_ZT, O_W1, O_OHT, O_G4, O_B2T = 0, 1024, 1280, 1536, 1540
C64 = 1541
# pk128a column offsets ([128, C128A]) -- small, loaded early
A_W2T, A_COEF, A_B1, A_BASE = 0, 128, 256, 258
C128A = 259
# pk128b column offsets ([128, C128B]) -- phase B constants
B_W2, B_GSEL, B_WOHND, B_COEFND, B_B2E, B_CBND, B_IDENT = (
    0, 1024, 1152, 1280, 1344, 1348, 1350)
B_MASK = 1478
C128B = 1494


def _stt_int(eng, out, in0, imm, in1, op0, op1):
    """scalar_tensor_tensor with an int32-typed immediate (bitvec ops)."""
    return eng.add_instruction(
        mybir.InstTensorScalarPtr(
            name=eng.bass.get_next_instruction_name(),
            is_scalar_tensor_tensor=True,
            op0=op0, op1=op1,
            ins=[eng.lower_ap(in0),
                 mybir.ImmediateValue(dtype=I32, value=imm),
                 eng.lower_ap(in1)],
            outs=[eng.lower_ap(out)]))


def _ts_int(eng, out, in0, imm, op0):
    """tensor_scalar with an int32-typed immediate (bitvec ops)."""
    return eng.add_instruction(
        mybir.InstTensorScalarPtr(
            name=eng.bass.get_next_instruction_name(),
            op0=op0,
            ins=[eng.lower_ap(in0),
                 mybir.ImmediateValue(dtype=I32, value=imm)],
            outs=[eng.lower_ap(out)]))


class _Bacc(bacc.Bacc):
    """Bacc that pins every activation to the one table holding
    {Relu, Exp, Ln, Copy}, avoiding per-switch ACT_TABLE_LOADs."""

    def insert_act_table_loads(self):
        has_act = any(isinstance(i, mybir.InstActivation)
                      for b in self.main_func.blocks for i in b.instructions)
        if not has_act:
            return
        tables = []
        for name, funcs in get_activation_tables(self.m.arch).items():
            keep = funcs if name == "natural_log_exp_and_others" else set()
            tables.append((name, keep))
        _br.insert_act_table_loads(self, tables)


def _build_nc():
    nc = _Bacc("TRN2", target_bir_lowering=False, num_devices=P)

    dp = nc.declare_dram_parameter
    pk64 = dp("pk64", [Lz, C64], F32R, isOutput=False)
    pk128a = dp("pk128a", [128, C128A], F32R, isOutput=False)
    pk128b = dp("pk128b", [128, C128B], F32R, isOutput=False)
    zr = dp("zr", [N, Lz], F32, isOutput=False)
    outp = dp("out", [BL], F32, isOutput=True)
    dbgi = dp("dbgids", [16, 32], I16, isOutput=True)

    with tile.TileContext(nc) as tc, ExitStack() as ctx:
        const = ctx.enter_context(tc.tile_pool(name="const", bufs=1))
        dram = ctx.enter_context(tc.tile_pool(name="dram", bufs=1, space="DRAM"))

        # phase-A-critical consts first on the sync queue; the big phase-B
        # tile goes through the gpsimd queue so its descriptors don't
        # contend with the phase-A loads.
        ka = const.tile([128, C128A], F32R, name="ka")
        nc.sync.dma_start(ka[:], pk128a[:])
        k64 = const.tile([Lz, C64], F32R, name="k64")
        nc.sync.dma_start(k64[:], pk64[:])
        # kb's DMA is emitted at the END of phase A: emitting it here would
        # fold its completion into phase-A wait thresholds (observed: relu
        # stalls until the 765KB load finishes).
        kb = const.tile([128, C128B], F32R, name="kb")

        def c64(off, w, p=Lz, dt=None):
            ap = k64[0:p, off:off + w]
            return ap.bitcast(dt) if dt else ap

        def ca(off, w, p=128, dt=None):
            ap = ka[0:p, off:off + w]
            return ap.bitcast(dt) if dt else ap

        def cb(off, w, p=128, dt=None):
            ap = kb[0:p, off:off + w]
            return ap.bitcast(dt) if dt else ap

        xin = dram.tile([B, 8], F32)
        xout = dram.tile([B, 8], F32)
        dscr = dram.tile([2 * BL, 512], F32)

        # local bin ids 0..1023 (shard recovered in phase B from slot pos)
        lid = const.tile([128, NL], I32, name="lid")
        nc.gpsimd.iota(lid[:], pattern=[[1, NL]], base=0, channel_multiplier=0)

        # dummy dma_gather + dummy activation: force the SWDGE library load
        # and the ACT table load to overlap the parameter DMAs instead of
        # stalling phase A/B.
        with ExitStack() as ctx0:
            pre = ctx0.enter_context(tc.tile_pool(name="pre", bufs=1))
            idxd = pre.tile([128, 8], I16, name="idxd")
            nc.vector.memset(idxd[:], 0)
            outd = pre.tile([128, 1, Lz], F32, name="outd")
            nc.gpsimd.dma_gather(outd[:], zr[:], idxd[:], num_idxs=128,
                                 num_idxs_reg=128, elem_size=Lz)
            ja = pre.tile([1, 2], F32, name="ja")
            nc.vector.memset(ja[:], 0)
            jb = pre.tile([1, 2], F32, name="jb")
            nc.scalar.activation(jb[:], ja[:], AF.Relu)

        # ================= phase A =================
        with ExitStack() as ctxA:
            pa = ctxA.enter_context(tc.tile_pool(name="pa", bufs=3, space="PSUM"))
            sp = ctxA.enter_context(tc.tile_pool(name="sp", bufs=1, space="PSUM"))
            act = ctxA.enter_context(tc.tile_pool(name="actA", bufs=1))
            scr = ctxA.enter_context(tc.tile_pool(name="scrA", bufs=2))

            # Full f32 on the score path (the top-16 must match the
            # reference exactly). All stages emit per-512-col halves so the
            # f=0 chain pipelines with f=1 across engines.
            hs = [act.tile([128, NL], F32, name=f"hh{m}") for m in range(2)]
            phs = [pa.tile([128, NL], F32, tag="mm", name=f"ph{m}")
                   for m in range(2)]
            pl3 = pa.tile([128, NL], F32, tag="mm")
            pse4 = sp.tile([4, NL], F32, tag="se")
            l3r = act.tile([NTAIL, NL], F32, name="l3r")
            e3r = act.tile([NTAIL, NL], F32, name="e3r")
            l4r = act.tile([4, NL], F32, name="l4r")
            # all relus before e3r/l4r so the ACT FIFO never blocks the
            # f=1 h-chain behind f=0's lse work
            for f in range(2):
                sl = slice(f * 512, (f + 1) * 512)
                for m in range(2):
                    nc.tensor.matmul(phs[m][:, sl],
                                     c64(O_W1 + m * 128, 128, dt=F32),
                                     c64(O_ZT + f * 512, 512, dt=F32),
                                     start=True, stop=True)
                    nc.scalar.activation(hs[m][:, sl], phs[m][:, sl], AF.Relu,
                                         bias=ca(A_B1 + m, 1, dt=F32))
            for f in range(2):
                sl = slice(f * 512, (f + 1) * 512)
                for kk in range(2):
                    nc.tensor.matmul(pl3[0:NTAIL, sl],
                                     ca(A_W2T + kk * NTAIL, NTAIL, dt=F32),
                                     hs[kk][:, sl],
                                     start=(kk == 0), stop=(kk == 1))
                nc.vector.tensor_copy(l3r[:, sl], pl3[0:NTAIL, sl])
                nc.scalar.activation(e3r[:, sl], pl3[0:NTAIL, sl], AF.Exp,
                                     bias=c64(O_B2T, 1, dt=F32))

            # scores scaled by 2^14 (folded into oht/coef on host), packed
            # as int32: clamp0(16384*s + 2^19) << 10 | local_id, top-8.
            # The oht partial sums only need l3r, so they run on the PE
            # while the ACT engine works through the exp/ln chain; the
            # accumulation groups close with the coef matmuls after l4r.
            psts = [pa.tile([128, NL], F32, tag="mm", name=f"pst{bt}")
                    for bt in range(2)]
            pks = [scr.tile([128, NL], I32, tag=f"pk{bt}", name=f"pk{bt}")
                   for bt in range(2)]
            for f in range(2):
                sl = slice(f * 512, (f + 1) * 512)
                for bt in range(2):
                    nc.tensor.matmul(psts[bt][:, sl],
                                     c64(O_OHT + bt * 128, 128, dt=F32),
                                     l3r[:, sl], start=True, stop=False)
            for f in range(2):
                sl = slice(f * 512, (f + 1) * 512)
                nc.tensor.matmul(pse4[:, sl], c64(O_G4, 4, dt=F32),
                                 e3r[:, sl], start=True, stop=True)
                nc.scalar.activation(l4r[:, sl], pse4[:, sl], AF.Ln)
            for f in range(2):
                sl = slice(f * 512, (f + 1) * 512)
                for bt in range(2):
                    nc.tensor.matmul(psts[bt][:, sl],
                                     ca(A_COEF, 128, p=4, dt=F32),
                                     l4r[:, sl], start=False, stop=True)
                    t32 = scr.tile([128, 512], I32, tag="t32")
                    nc.vector.tensor_scalar(t32[:], psts[bt][:, sl],
                                            524288.0, 0.0,
                                            op0=ALU.add, op1=ALU.max)
                    _stt_int(nc.vector, pks[bt][:, sl], t32[:], 10,
                             lid[:, sl], ALU.logical_shift_left,
                             ALU.bitwise_or)
            for bt in range(2):
                x_sb = act.tile([128, 8], F32, name=f"x{bt}")
                nc.vector.max(x_sb[:], pks[bt][:].bitcast(F32))
                nc.sync.dma_start(xin[bt * 128:(bt + 1) * 128, :], x_sb[:])

            # phase-B constants load now, overlapping the collective wait.
            # Issued from the Scalar queue: it sits behind all phase-A ACT
            # work, so its completion can't leak into phase-A DMA-semaphore
            # thresholds (a gpsimd-issued load stalled relu until ~24us).
            nc.scalar.dma_start(kb[:], pk128b[:])

        nc.gpsimd.collective_compute(
            "AllToAll", ALU.bypass, replica_groups=[list(range(P))],
            ins=[xin[:].opt()], outs=[xout[:].opt()],
        )

        # ================= phase B =================
        with ExitStack() as ctxB:
            pb = ctxB.enter_context(tc.tile_pool(name="pb", bufs=4, space="PSUM"))
            spb = ctxB.enter_context(tc.tile_pool(name="spb", bufs=1, space="PSUM"))
            ptr = ctxB.enter_context(tc.tile_pool(name="ptr", bufs=1, space="PSUM"))
            pnd_p = ctxB.enter_context(tc.tile_pool(name="pnd", bufs=1, space="PSUM"))
            act = ctxB.enter_context(tc.tile_pool(name="actB", bufs=1))

            # idx tile for dma_gather: only partitions 0:16 carry indices
            # (the SWDGE gather reads the 16-partition-wrapped layout);
            # zero the rest off the critical path.
            idxall = act.tile([128, 32], I16, name="idxall")
            nc.vector.memset(idxall[:], 0)

            # 64 packed candidates per local batch row
            cands = act.tile([BL, P * 8], F32, name="cands")
            nc.sync.dma_start(cands[:],
                              xout[:].rearrange("(s p) f -> p s f", s=P))
            wv = act.tile([BL, 16], F32, name="wv")
            nc.vector.max(wv[:, 0:8], cands[:])
            cm = act.tile([BL, P * 8], F32, name="cm")
            nc.vector.match_replace(cm[:], wv[:, 0:8], cands[:], 0.0)
            nc.vector.max(wv[:, 8:16], cm[:])

            # winner global ids: shard from slot position (pos//8)*1024,
            # local id from the packed low 10 bits
            ids32 = act.tile([32, 32], I32, name="ids32")
            nc.vector.memset(ids32[:], 0)
            pu = act.tile([BL, 16], mybir.dt.uint32, name="pu")
            nc.vector.max_index(pu[:, 0:8], wv[:, 0:8], cands[:])
            nc.vector.max_index(pu[:, 8:16], wv[:, 8:16], cands[:])
            m1 = act.tile([BL, 16], I32, name="m1")
            _ts_int(nc.vector, m1[:], pu[:].bitcast(I32), 7,
                    ALU.logical_shift_left)
            m2 = act.tile([BL, 16], I32, name="m2")
            _ts_int(nc.vector, m2[:], m1[:], 0x1C00, ALU.bitwise_and)
            _stt_int(nc.vector, ids32[:, 0:16], wv[:].bitcast(I32), 1023,
                     m2[:], ALU.bitwise_and, ALU.bitwise_or)
            idT = act.tile([32, 32], I32, name="idT")
            nc.vector.transpose(idT[:], ids32[:])
            nc.vector.tensor_copy(idxall[0:16, :], idT[0:16, :])
            for g in [1, 2, 4]:
                nc.sync.dma_start(idxall[16 * g:16 * 2 * g, :],
                                  idxall[0:16 * g, :])
            nc.sync.dma_start(dbgi[:], idxall[0:16, :])

            # gather winner z rows from DRAM: ztop[p, i, :] = z[idx[i*128+p]]
            ztop = act.tile([128, 4, Lz], F32, name="ztop")
            nc.gpsimd.dma_gather(ztop[:], zr[:], idxall[:], num_idxs=512,
                                 num_idxs_reg=512, elem_size=Lz)
            # transpose to [64, 512] via PE
            pzt = ptr.tile([Lz, 512], F32, tag="tr")
            for i in range(4):
                nc.tensor.transpose(pzt[:, i * 128:(i + 1) * 128],
                                    ztop[:, i, :].bitcast(F32),
                                    cb(B_IDENT, 128, dt=F32))
            ztr = act.tile([Lz, 512], F32R, name="ztr")
            nc.vector.tensor_copy(ztr[:], pzt[:])

            # h2 = relu(W1.T @ ztop + b1)
            h2s = []
            for m in range(2):
                ph2 = pb.tile([128, 512], F32, tag="mmb")
                nc.tensor.matmul(ph2[:], c64(O_W1 + m * 128, 128), ztr[:],
                                 start=True, stop=True)
                hh2 = act.tile([128, 512], F32R, name=f"hh2{m}")
                nc.scalar.activation(hh2[:], ph2[:], AF.Relu,
                                     bias=ca(A_B1 + m, 1, dt=F32))
                h2s.append(hh2)

            # num (rows 0-31) and den (rows 32-63): the woh part only needs
            # h2, so it runs ahead of the l2 loop; the accumulation group
            # stays open until the coefnd matmul after lgr.
            pnd = pnd_p.tile([2 * BL, 512], F32, tag="nd")
            for kk in range(2):
                nc.tensor.matmul(pnd[:], cb(B_WOHND + kk * 64, 64), h2s[kk][:],
                                 start=(kk == 0), stop=False)

            # full logits per t-tile -> exp -> per-feature sums
            pse2 = spb.tile([32, 512], F32, tag="seb")
            for t in range(4):
                pl2 = pb.tile([128, 512], F32, tag="mmb")
                for kk in range(2):
                    nc.tensor.matmul(pl2[:], cb(B_W2 + (kk * 4 + t) * 128, 128),
                                     h2s[kk][:], start=(kk == 0), stop=(kk == 1))
                e2r = act.tile([128, 512], F32R, name=f"e2r{t}")
                nc.scalar.activation(e2r[:], pl2[:], AF.Exp,
                                     bias=cb(B_B2E + t, 1, dt=F32))
                nc.tensor.matmul(pse2[:], cb(B_GSEL + t * 32, 32), e2r[:],
                                 start=(t == 0), stop=(t == 3))
            lgr = act.tile([32, 512], F32R, name="lgr")
            nc.scalar.activation(lgr[:], pse2[:], AF.Ln)

            nc.tensor.matmul(pnd[:], cb(B_COEFND, 64, p=32), lgr[:],
                             start=False, stop=True)
            ndf = act.tile([2 * BL, 512], F32, name="ndf")
            nc.scalar.copy(ndf[:], pnd[:])
            nc.sync.dma_start(dscr[:], ndf[:])

            # diagonal extraction, both halves in one DMA:
            # numd[b, half*16+j] = ndf[half*32+b, b*16+j]
            numd = act.tile([BL, 32], F32, name="numd")
            dnd = bass.AP(tensor=dscr[:].tensor, offset=0,
                          ap=[[512 + 16, BL], [BL * 512, 2], [1, 16]])
            nc.sync.dma_start(numd[:], dnd)

            # logsumexp(num) - logsumexp(den)
            ng = act.tile([BL, 2], F32, name="ng")
            nc.vector.tensor_reduce(ng[:, 0:1], numd[:, 0:16], axis=AX.X,
                                    op=ALU.max, negate=True)
            nc.vector.tensor_reduce(ng[:, 1:2], numd[:, 16:32], axis=AX.X,
                                    op=ALU.max, negate=True)
            # t2 only needs ng: compute it now, off the exp/ln chain
            t2 = act.tile([BL, 1], F32, name="t2")
            nc.vector.tensor_sub(t2[:], ng[:, 1:2], ng[:, 0:1])
            s2 = act.tile([BL, 2], F32, name="s2")
            en = act.tile([BL, 16], F32, name="en")
            nc.scalar.activation(en[:], numd[:, 0:16], AF.Exp,
                                 bias=ng[:, 0:1], accum_out=s2[:, 0:1])
            ed = act.tile([BL, 16], F32, name="ed")
            nc.scalar.activation(ed[:], numd[:, 16:32], AF.Exp,
                                 bias=ng[:, 1:2], accum_out=s2[:, 1:2])
            lg = act.tile([BL, 2], F32, name="lg")
            nc.scalar.activation(lg[:], s2[:], AF.Ln)
            t1 = act.tile([BL, 1], F32, name="t1")
            nc.vector.tensor_sub(t1[:], lg[:, 0:1], lg[:, 1:2])
            # + (oh.b2 - oh_obs.b2) fold (zero for this model, kept general)
            t3 = act.tile([BL, 1], F32, name="t3")
            nc.vector.scalar_tensor_tensor(
                t3[:], t1[:], cb(B_CBND, 1, p=BL, dt=F32), t2[:],
                op0=ALU.add, op1=ALU.add)
            nc.sync.dma_start(outp[:], t3[:, 0])

    nc.compile()
    return nc


def _host_prep(x, z, W1, b1, W2, b2):
    oh = np.zeros((B, DC), np.float32)
    oh[np.arange(B)[:, None], np.arange(D)[None, :] * C + x] = 1.0
    oh_obs = oh.copy()
    oh_obs[:, DC - 4 * C:] = 0.0
    woh = oh @ W2.T          # (256, 256)
    wohd = oh_obs @ W2.T
    cbn = oh @ b2            # (256,)
    cbd = oh_obs @ b2

    k64c = np.zeros((Lz, C64), np.float32)
    k64c[:, O_W1:O_W1 + H] = W1
    # tail one-hot transposed, scaled by 2^14 for score packing
    k64c[:, O_OHT:O_OHT + B] = 16384.0 * oh[:, DC - NTAIL:].T
    g4 = np.zeros((Lz, 4), np.float32)
    g4[np.arange(Lz), np.arange(Lz) // C] = 1.0
    k64c[:, O_G4:O_G4 + 4] = g4
    k64c[:, O_B2T] = b2[DC - NTAIL:]

    kac = np.zeros((128, C128A), np.float32)
    for kk in range(2):
        kac[:, A_W2T + kk * NTAIL:A_W2T + (kk + 1) * NTAIL] = \
            W2[kk * 128:(kk + 1) * 128, DC - NTAIL:]
    kac[0:4, A_COEF:A_COEF + 128] = -16384.0
    kac[:, A_B1:A_B1 + 2] = b1.reshape(2, 128).T

    kbc = np.zeros((128, C128B), np.float32)
    for kk in range(2):
        for t in range(4):
            kbc[:, B_W2 + (kk * 4 + t) * 128:B_W2 + (kk * 4 + t + 1) * 128] = \
                W2[kk * 128:(kk + 1) * 128, t * 128:(t + 1) * 128]
    p_idx = np.arange(128)
    for t in range(4):
        kbc[p_idx, B_GSEL + t * 32 + t * 8 + p_idx // C] = 1.0
    kbc[0:32, B_COEFND:B_COEFND + 32] = -1.0
    kbc[0:28, B_COEFND + 32:B_COEFND + 64] = -1.0
    kbc[:, B_B2E:B_B2E + 4] = b2.reshape(4, 128).T
    kbc[:, B_IDENT:B_IDENT + 128] = np.eye(128, dtype=np.float32)
    kbc[:, B_MASK:B_MASK + 16] = np.full(
        (128, 16), np.int32(0x1C00)).view(np.float32)

    zf = np.ascontiguousarray(z, np.float32)
    in_maps = []
    for c in range(P):
        kc64 = k64c.copy()
        kc64[:, O_ZT:O_ZT + NL] = z[c * NL:(c + 1) * NL, :].T
        kca = kac.copy()
        kcb = kbc.copy()
        bsl = slice(c * BL, (c + 1) * BL)
        for kk in range(2):
            kcb[:, B_WOHND + kk * 64:B_WOHND + kk * 64 + BL] = \
                woh[bsl, kk * 128:(kk + 1) * 128].T
            kcb[:, B_WOHND + kk * 64 + BL:B_WOHND + (kk + 1) * 64] = \
                wohd[bsl, kk * 128:(kk + 1) * 128].T
        kcb[0:BL, B_CBND] = cbn[bsl] - cbd[bsl]
        in_maps.append(dict(pk64=kc64, pk128a=kca, pk128b=kcb, zr=zf))
    return in_maps


_NC_CACHE = {}


def kernel(x, log_w, z, k, W1, b1, W2, b2, _trace=False, _trace_kwargs=None):
    assert int(k) == K
    in_maps = _host_prep(np.asarray(x, np.int32), np.asarray(z, np.float32),
                         np.asarray(W1, np.float32), np.asarray(b1, np.float32),
                         np.asarray(W2, np.float32), np.asarray(b2, np.float32))
    if "nc" not in _NC_CACHE:
        _NC_CACHE["nc"] = _build_nc()
    nc = _NC_CACHE["nc"]
    res = run_bass_kernel_spmd(
        nc, in_maps, list(range(P)), trace=_trace, **(_trace_kwargs or {}))
    if _trace:
        _NC_CACHE["last_result"] = res
    return np.concatenate([np.asarray(res.results[c]["out"], np.float32)
                           for c in range(P)])


# revision 49
# speedup vs baseline: 1.1301x; 1.1301x over previous
"""Trainium2 Bass kernel v3 for nn_CategoricalDecoder (topk_masking).

Phase A (bin-sharded, single-term f32r): tail-feature logits for the local
1024-bin shard, scores packed as int32 (score<<13 | global bin id), local
top-8 per batch row via max8. AllToAll flips to batch sharding (8KB).
Phase B: merge 64 candidates/row -> top-16 packed (ids come free via
bitwise AND), dma_gather of winner z rows from DRAM, single-term f32r
recompute of num/den (den exact via host-folded oh@W2 matmuls), logsumexp.
"""

import numpy as np
from contextlib import ExitStack

import bass_rust as _br
import concourse.bass as bass
import concourse.bacc as bacc
import concourse.tile as tile
from concourse import mybir
from concourse.bass_utils import run_bass_kernel_spmd
from concourse.hw_specs import get_activation_tables

F32 = mybir.dt.float32
F32R = mybir.dt.float32r
I32 = mybir.dt.int32
I16 = mybir.dt.int16
AF = mybir.ActivationFunctionType
ALU = mybir.AluOpType
AX = mybir.AxisListType

B, N, Lz, H, D, C = 256, 8192, 64, 256, 32, 16
DC = D * C
P = 8
NL = N // P
BL = B // P
K = 16
NTAIL = 64  # tail-feature logit rows (4 features x 16 classes)

# pk64 column offsets ([64, C64])
O_ZT, O_W1, O_OHT, O_G4, O_B2T = 0, 1024, 1280, 1536, 1540
C64 = 1541
# pk128a column offsets ([128, C128A]) -- small, loaded early
A_W2T, A_COEF, A_B1, A_BASE = 0, 128, 256, 258
C128A = 259
# pk128b column offsets ([128, C128B]) -- phase B constants
B_W2, B_GSEL, B_WOHND, B_COEFND, B_B2E, B_CBND, B_IDENT = (
    0, 1024, 1152, 1280, 1344, 1348, 1350)
B_MASK = 1478
C128B = 1494


def _stt_int(eng, out, in0, imm, in1, op0, op1):
    """scalar_tensor_tensor with an int32-typed immediate (bitvec ops)."""
    return eng.add_instruction(
        mybir.InstTensorScalarPtr(
            name=eng.bass.get_next_instruction_name(),
            is_scalar_tensor_tensor=True,
            op0=op0, op1=op1,
            ins=[eng.lower_ap(in0),
                 mybir.ImmediateValue(dtype=I32, value=imm),
                 eng.lower_ap(in1)],
            outs=[eng.lower_ap(out)]))


def _ts_int(eng, out, in0, imm, op0):
    """tensor_scalar with an int32-typed immediate (bitvec ops)."""
    return eng.add_instruction(
        mybir.InstTensorScalarPtr(
            name=eng.bass.get_next_instruction_name(),
            op0=op0,
            ins=[eng.lower_ap(in0),
                 mybir.ImmediateValue(dtype=I32, value=imm)],
            outs=[eng.lower_ap(out)]))


class _Bacc(bacc.Bacc):
    """Bacc that pins every activation to the one table holding
    {Relu, Exp, Ln, Copy}, avoiding per-switch ACT_TABLE_LOADs."""

    def insert_act_table_loads(self):
        has_act = any(isinstance(i, mybir.InstActivation)
                      for b in self.main_func.blocks for i in b.instructions)
        if not has_act:
            return
        tables = []
        for name, funcs in get_activation_tables(self.m.arch).items():
            keep = funcs if name == "natural_log_exp_and_others" else set()
            tables.append((name, keep))
        _br.insert_act_table_loads(self, tables)


def _build_nc():
    nc = _Bacc("TRN2", target_bir_lowering=False, num_devices=P)

    dp = nc.declare_dram_parameter
    pk64 = dp("pk64", [Lz, C64], F32R, isOutput=False)
    pk128a = dp("pk128a", [128, C128A], F32R, isOutput=False)
    pk128b = dp("pk128b", [128, C128B], F32R, isOutput=False)
    zr = dp("zr", [N, Lz], F32, isOutput=False)
    outp = dp("out", [BL], F32, isOutput=True)
    dbgi = dp("dbgids", [16, 32], I16, isOutput=True)

    with tile.TileContext(nc) as tc, ExitStack() as ctx:
        const = ctx.enter_context(tc.tile_pool(name="const", bufs=1))
        dram = ctx.enter_context(tc.tile_pool(name="dram", bufs=1, space="DRAM"))

        # phase-A-critical consts first on the sync queue; the big phase-B
        # tile goes through the gpsimd queue so its descriptors don't
        # contend with the phase-A loads.
        ka = const.tile([128, C128A], F32R, name="ka")
        nc.sync.dma_start(ka[:], pk128a[:])
        k64 = const.tile([Lz, C64], F32R, name="k64")
        nc.sync.dma_start(k64[:], pk64[:])
        # kb's DMA is emitted at the END of phase A: emitting it here would
        # fold its completion into phase-A wait thresholds (observed: relu
        # stalls until the 765KB load finishes).
        kb = const.tile([128, C128B], F32R, name="kb")

        def c64(off, w, p=Lz, dt=None):
            ap = k64[0:p, off:off + w]
            return ap.bitcast(dt) if dt else ap

        def ca(off, w, p=128, dt=None):
            ap = ka[0:p, off:off + w]
            return ap.bitcast(dt) if dt else ap

        def cb(off, w, p=128, dt=None):
            ap = kb[0:p, off:off + w]
            return ap.bitcast(dt) if dt else ap

        xin = dram.tile([B, 8], F32)
        xout = dram.tile([B, 8], F32)
        dscr = dram.tile([2 * BL, 512], F32)

        # local bin ids 0..1023 (shard recovered in phase B from slot pos)
        lid = const.tile([128, NL], I32, name="lid")
        nc.gpsimd.iota(lid[:], pattern=[[1, NL]], base=0, channel_multiplier=0)

        # dummy dma_gather + dummy activation: force the SWDGE library load
        # and the ACT table load to overlap the parameter DMAs instead of
        # stalling phase A/B.
        with ExitStack() as ctx0:
            pre = ctx0.enter_context(tc.tile_pool(name="pre", bufs=1))
            idxd = pre.tile([128, 8], I16, name="idxd")
            nc.vector.memset(idxd[:], 0)
            outd = pre.tile([128, 1, Lz], F32, name="outd")
            nc.gpsimd.dma_gather(outd[:], zr[:], idxd[:], num_idxs=128,
                                 num_idxs_reg=128, elem_size=Lz)
            ja = pre.tile([1, 2], F32, name="ja")
            nc.vector.memset(ja[:], 0)
            jb = pre.tile([1, 2], F32, name="jb")
            nc.scalar.activation(jb[:], ja[:], AF.Relu)

        # ================= phase A =================
        with ExitStack() as ctxA:
            pa = ctxA.enter_context(tc.tile_pool(name="pa", bufs=3, space="PSUM"))
            sp = ctxA.enter_context(tc.tile_pool(name="sp", bufs=1, space="PSUM"))
            act = ctxA.enter_context(tc.tile_pool(name="actA", bufs=1))
            scr = ctxA.enter_context(tc.tile_pool(name="scrA", bufs=2))

            # Full f32 on the score path (the top-16 must match the
            # reference exactly). All stages emit per-512-col halves so the
            # f=0 chain pipelines with f=1 across engines.
            hs = [act.tile([128, NL], F32, name=f"hh{m}") for m in range(2)]
            phs = [pa.tile([128, NL], F32, tag="mm", name=f"ph{m}")
                   for m in range(2)]
            pl3 = pa.tile([128, NL], F32, tag="mm")
            pse4 = sp.tile([4, NL], F32, tag="se")
            l3r = act.tile([NTAIL, NL], F32, name="l3r")
            e3r = act.tile([NTAIL, NL], F32, name="e3r")
            l4r = act.tile([4, NL], F32, name="l4r")
            # all relus before e3r/l4r so the ACT FIFO never blocks the
            # f=1 h-chain behind f=0's lse work
            for f in range(2):
                sl = slice(f * 512, (f + 1) * 512)
                for m in range(2):
                    nc.tensor.matmul(phs[m][:, sl],
                                     c64(O_W1 + m * 128, 128, dt=F32),
                                     c64(O_ZT + f * 512, 512, dt=F32),
                                     start=True, stop=True)
                    nc.scalar.activation(hs[m][:, sl], phs[m][:, sl], AF.Relu,
                                         bias=ca(A_B1 + m, 1, dt=F32))
            for f in range(2):
                sl = slice(f * 512, (f + 1) * 512)
                for kk in range(2):
                    nc.tensor.matmul(pl3[0:NTAIL, sl],
                                     ca(A_W2T + kk * NTAIL, NTAIL, dt=F32),
                                     hs[kk][:, sl],
                                     start=(kk == 0), stop=(kk == 1))
                nc.vector.tensor_copy(l3r[:, sl], pl3[0:NTAIL, sl])
                nc.scalar.activation(e3r[:, sl], pl3[0:NTAIL, sl], AF.Exp,
                                     bias=c64(O_B2T, 1, dt=F32))

            # scores scaled by 2^14 (folded into oht/coef on host), packed
            # as int32: clamp0(16384*s + 2^19) << 10 | local_id, top-8.
            # The oht partial sums only need l3r, so they run on the PE
            # while the ACT engine works through the exp/ln chain; the
            # accumulation groups close with the coef matmuls after l4r.
            psts = [pa.tile([128, NL], F32, tag="mm", name=f"pst{bt}")
                    for bt in range(2)]
            pks = [scr.tile([128, NL], I32, tag=f"pk{bt}", name=f"pk{bt}")
                   for bt in range(2)]
            for f in range(2):
                sl = slice(f * 512, (f + 1) * 512)
                for bt in range(2):
                    nc.tensor.matmul(psts[bt][:, sl],
                                     c64(O_OHT + bt * 128, 128, dt=F32),
                                     l3r[:, sl], start=True, stop=False)
            for f in range(2):
                sl = slice(f * 512, (f + 1) * 512)
                nc.tensor.matmul(pse4[:, sl], c64(O_G4, 4, dt=F32),
                                 e3r[:, sl], start=True, stop=True)
                nc.scalar.activation(l4r[:, sl], pse4[:, sl], AF.Ln)
            for f in range(2):
                sl = slice(f * 512, (f + 1) * 512)
                for bt in range(2):
                    nc.tensor.matmul(psts[bt][:, sl],
                                     ca(A_COEF, 128, p=4, dt=F32),
                                     l4r[:, sl], start=False, stop=True)
                    t32 = scr.tile([128, 512], I32, tag="t32")
                    nc.vector.tensor_scalar(t32[:], psts[bt][:, sl],
                                            524288.0, 0.0,
                                            op0=ALU.add, op1=ALU.max)
                    _stt_int(nc.vector, pks[bt][:, sl], t32[:], 10,
                             lid[:, sl], ALU.logical_shift_left,
                             ALU.bitwise_or)
            for bt in range(2):
                x_sb = act.tile([128, 8], F32, name=f"x{bt}")
                nc.vector.max(x_sb[:], pks[bt][:].bitcast(F32))
                nc.sync.dma_start(xin[bt * 128:(bt + 1) * 128, :], x_sb[:])

            # phase-B constants load now, overlapping the collective wait.
            # Issued from the Scalar queue: it sits behind all phase-A ACT
            # work, so its completion can't leak into phase-A DMA-semaphore
            # thresholds (a gpsimd-issued load stalled relu until ~24us).
            nc.scalar.dma_start(kb[:], pk128b[:])

        nc.gpsimd.collective_compute(
            "AllToAll", ALU.bypass, replica_groups=[list(range(P))],
            ins=[xin[:].opt()], outs=[xout[:].opt()],
        )

        # ================= phase B =================
        with ExitStack() as ctxB:
            pb = ctxB.enter_context(tc.tile_pool(name="pb", bufs=4, space="PSUM"))
            spb = ctxB.enter_context(tc.tile_pool(name="spb", bufs=1, space="PSUM"))
            ptr = ctxB.enter_context(tc.tile_pool(name="ptr", bufs=1, space="PSUM"))
            pnd_p = ctxB.enter_context(tc.tile_pool(name="pnd", bufs=1, space="PSUM"))
            act = ctxB.enter_context(tc.tile_pool(name="actB", bufs=1))

            # idx tile for dma_gather: only partitions 0:16 carry indices
            # (the SWDGE gather reads the 16-partition-wrapped layout);
            # zero the rest off the critical path.
            idxall = act.tile([128, 32], I16, name="idxall")
            nc.vector.memset(idxall[:], 0)

            # 64 packed candidates per local batch row
            cands = act.tile([BL, P * 8], F32, name="cands")
            nc.sync.dma_start(cands[:],
                              xout[:].rearrange("(s p) f -> p s f", s=P))
            wv = act.tile([BL, 16], F32, name="wv")
            pu = act.tile([BL, 16], mybir.dt.uint32, name="pu")
            nc.vector.max(wv[:, 0:8], cands[:])
            nc.vector.max_index(pu[:, 0:8], wv[:, 0:8], cands[:])
            cm = act.tile([BL, P * 8], F32, name="cm")
            nc.vector.match_replace(cm[:], wv[:, 0:8], cands[:], 0.0)
            nc.vector.max(wv[:, 8:16], cm[:])

            # winner global ids: shard from slot position (pos//8)*1024,
            # local id from the packed low 10 bits
            ids32 = act.tile([32, 32], I32, name="ids32")
            nc.vector.memset(ids32[:], 0)
            nc.vector.max_index(pu[:, 8:16], wv[:, 8:16], cands[:])
            m1 = act.tile([BL, 16], I32, name="m1")
            _ts_int(nc.vector, m1[:], pu[:].bitcast(I32), 7,
                    ALU.logical_shift_left)
            m2 = act.tile([BL, 16], I32, name="m2")
            _ts_int(nc.vector, m2[:], m1[:], 0x1C00, ALU.bitwise_and)
            _stt_int(nc.vector, ids32[:, 0:16], wv[:].bitcast(I32), 1023,
                     m2[:], ALU.bitwise_and, ALU.bitwise_or)
            idT = act.tile([32, 32], I32, name="idT")
            nc.vector.transpose(idT[:], ids32[:])
            nc.vector.tensor_copy(idxall[0:16, :], idT[0:16, :])
            for g in [1, 2, 4]:
                nc.sync.dma_start(idxall[16 * g:16 * 2 * g, :],
                                  idxall[0:16 * g, :])
            nc.sync.dma_start(dbgi[:], idxall[0:16, :])

            # gather winner z rows from DRAM: ztop[p, i, :] = z[idx[i*128+p]]
            ztop = act.tile([128, 4, Lz], F32, name="ztop")
            nc.gpsimd.dma_gather(ztop[:], zr[:], idxall[:], num_idxs=512,
                                 num_idxs_reg=512, elem_size=Lz)
            # transpose to [64, 512] via PE
            pzt = ptr.tile([Lz, 512], F32, tag="tr")
            for i in range(4):
                nc.tensor.transpose(pzt[:, i * 128:(i + 1) * 128],
                                    ztop[:, i, :].bitcast(F32),
                                    cb(B_IDENT, 128, dt=F32))
            ztr = act.tile([Lz, 512], F32R, name="ztr")
            nc.vector.tensor_copy(ztr[:], pzt[:])

            # h2 = relu(W1.T @ ztop + b1)
            h2s = []
            for m in range(2):
                ph2 = pb.tile([128, 512], F32, tag="mmb")
                nc.tensor.matmul(ph2[:], c64(O_W1 + m * 128, 128), ztr[:],
                                 start=True, stop=True)
                hh2 = act.tile([128, 512], F32R, name=f"hh2{m}")
                nc.scalar.activation(hh2[:], ph2[:], AF.Relu,
                                     bias=ca(A_B1 + m, 1, dt=F32))
                h2s.append(hh2)

            # num (rows 0-31) and den (rows 32-63): the woh part only needs
            # h2, so it runs ahead of the l2 loop; the accumulation group
            # stays open until the coefnd matmul after lgr.
            pnd = pnd_p.tile([2 * BL, 512], F32, tag="nd")
            for kk in range(2):
                nc.tensor.matmul(pnd[:], cb(B_WOHND + kk * 64, 64), h2s[kk][:],
                                 start=(kk == 0), stop=False)

            # full logits per t-tile -> exp -> per-feature sums
            pse2 = spb.tile([32, 512], F32, tag="seb")
            for t in range(4):
                pl2 = pb.tile([128, 512], F32, tag="mmb")
                for kk in range(2):
                    nc.tensor.matmul(pl2[:], cb(B_W2 + (kk * 4 + t) * 128, 128),
                                     h2s[kk][:], start=(kk == 0), stop=(kk == 1))
                e2r = act.tile([128, 512], F32R, name=f"e2r{t}")
                nc.scalar.activation(e2r[:], pl2[:], AF.Exp,
                                     bias=cb(B_B2E + t, 1, dt=F32))
                nc.tensor.matmul(pse2[:], cb(B_GSEL + t * 32, 32), e2r[:],
                                 start=(t == 0), stop=(t == 3))
            lgr = act.tile([32, 512], F32R, name="lgr")
            nc.scalar.activation(lgr[:], pse2[:], AF.Ln)

            nc.tensor.matmul(pnd[:], cb(B_COEFND, 64, p=32), lgr[:],
                             start=False, stop=True)
            ndf = act.tile([2 * BL, 512], F32, name="ndf")
            nc.scalar.copy(ndf[:], pnd[:])
            nc.sync.dma_start(dscr[:], ndf[:])

            # diagonal extraction, both halves in one DMA:
            # numd[b, half*16+j] = ndf[half*32+b, b*16+j]
            numd = act.tile([BL, 32], F32, name="numd")
            dnd = bass.AP(tensor=dscr[:].tensor, offset=0,
                          ap=[[512 + 16, BL], [BL * 512, 2], [1, 16]])
            nc.sync.dma_start(numd[:], dnd)

            # logsumexp(num) - logsumexp(den)
            ng = act.tile([BL, 2], F32, name="ng")
            nc.vector.tensor_reduce(ng[:, 0:1], numd[:, 0:16], axis=AX.X,
                                    op=ALU.max, negate=True)
            nc.vector.tensor_reduce(ng[:, 1:2], numd[:, 16:32], axis=AX.X,
                                    op=ALU.max, negate=True)
            # t2 only needs ng: compute it now, off the exp/ln chain
            t2 = act.tile([BL, 1], F32, name="t2")
            nc.vector.tensor_sub(t2[:], ng[:, 1:2], ng[:, 0:1])
            s2 = act.tile([BL, 2], F32, name="s2")
            en = act.tile([BL, 16], F32, name="en")
            nc.scalar.activation(en[:], numd[:, 0:16], AF.Exp,
                                 bias=ng[:, 0:1], accum_out=s2[:, 0:1])
            ed = act.tile([BL, 16], F32, name="ed")
            nc.scalar.activation(ed[:], numd[:, 16:32], AF.Exp,
                                 bias=ng[:, 1:2], accum_out=s2[:, 1:2])
            lg = act.tile([BL, 2], F32, name="lg")
            nc.scalar.activation(lg[:], s2[:], AF.Ln)
            t1 = act.tile([BL, 1], F32, name="t1")
            nc.vector.tensor_sub(t1[:], lg[:, 0:1], lg[:, 1:2])
            # + (oh.b2 - oh_obs.b2) fold (zero for this model, kept general)
            t3 = act.tile([BL, 1], F32, name="t3")
            nc.vector.scalar_tensor_tensor(
                t3[:], t1[:], cb(B_CBND, 1, p=BL, dt=F32), t2[:],
                op0=ALU.add, op1=ALU.add)
            nc.sync.dma_start(outp[:], t3[:, 0])

    nc.compile()
    return nc


def _host_prep(x, z, W1, b1, W2, b2):
    oh = np.zeros((B, DC), np.float32)
    oh[np.arange(B)[:, None], np.arange(D)[None, :] * C + x] = 1.0
    oh_obs = oh.copy()
    oh_obs[:, DC - 4 * C:] = 0.0
    woh = oh @ W2.T          # (256, 256)
    wohd = oh_obs @ W2.T
    cbn = oh @ b2            # (256,)
    cbd = oh_obs @ b2

    k64c = np.zeros((Lz, C64), np.float32)
    k64c[:, O_W1:O_W1 + H] = W1
    # tail one-hot transposed, scaled by 2^14 for score packing
    k64c[:, O_OHT:O_OHT + B] = 16384.0 * oh[:, DC - NTAIL:].T
    g4 = np.zeros((Lz, 4), np.float32)
    g4[np.arange(Lz), np.arange(Lz) // C] = 1.0
    k64c[:, O_G4:O_G4 + 4] = g4
    k64c[:, O_B2T] = b2[DC - NTAIL:]

    kac = np.zeros((128, C128A), np.float32)
    for kk in range(2):
        kac[:, A_W2T + kk * NTAIL:A_W2T + (kk + 1) * NTAIL] = \
            W2[kk * 128:(kk + 1) * 128, DC - NTAIL:]
    kac[0:4, A_COEF:A_COEF + 128] = -16384.0
    kac[:, A_B1:A_B1 + 2] = b1.reshape(2, 128).T

    kbc = np.zeros((128, C128B), np.float32)
    for kk in range(2):
        for t in range(4):
            kbc[:, B_W2 + (kk * 4 + t) * 128:B_W2 + (kk * 4 + t + 1) * 128] = \
                W2[kk * 128:(kk + 1) * 128, t * 128:(t + 1) * 128]
    p_idx = np.arange(128)
    for t in range(4):
        kbc[p_idx, B_GSEL + t * 32 + t * 8 + p_idx // C] = 1.0
    kbc[0:32, B_COEFND:B_COEFND + 32] = -1.0
    kbc[0:28, B_COEFND + 32:B_COEFND + 64] = -1.0
    kbc[:, B_B2E:B_B2E + 4] = b2.reshape(4, 128).T
    kbc[:, B_IDENT:B_IDENT + 128] = np.eye(128, dtype=np.float32)
    kbc[:, B_MASK:B_MASK + 16] = np.full(
        (128, 16), np.int32(0x1C00)).view(np.float32)

    zf = np.ascontiguousarray(z, np.float32)
    in_maps = []
    for c in range(P):
        kc64 = k64c.copy()
        kc64[:, O_ZT:O_ZT + NL] = z[c * NL:(c + 1) * NL, :].T
        kca = kac.copy()
        kcb = kbc.copy()
        bsl = slice(c * BL, (c + 1) * BL)
        for kk in range(2):
            kcb[:, B_WOHND + kk * 64:B_WOHND + kk * 64 + BL] = \
                woh[bsl, kk * 128:(kk + 1) * 128].T
            kcb[:, B_WOHND + kk * 64 + BL:B_WOHND + (kk + 1) * 64] = \
                wohd[bsl, kk * 128:(kk + 1) * 128].T
        kcb[0:BL, B_CBND] = cbn[bsl] - cbd[bsl]
        in_maps.append(dict(pk64=kc64, pk128a=kca, pk128b=kcb, zr=zf))
    return in_maps


_NC_CACHE = {}


def kernel(x, log_w, z, k, W1, b1, W2, b2, _trace=False, _trace_kwargs=None):
    assert int(k) == K
    in_maps = _host_prep(np.asarray(x, np.int32), np.asarray(z, np.float32),
                         np.asarray(W1, np.float32), np.asarray(b1, np.float32),
                         np.asarray(W2, np.float32), np.asarray(b2, np.float32))
    if "nc" not in _NC_CACHE:
        _NC_CACHE["nc"] = _build_nc()
    nc = _NC_CACHE["nc"]
    res = run_bass_kernel_spmd(
        nc, in_maps, list(range(P)), trace=_trace, **(_trace_kwargs or {}))
    if _trace:
        _NC_CACHE["last_result"] = res
    return np.concatenate([np.asarray(res.results[c]["out"], np.float32)
                           for c in range(P)])
